# revision 5
# baseline (speedup 1.0000x reference)
"""Causal self-attention (B=4, T=2048, H=768, NH=12) on 8 trn2 cores.

Sharding: core c -> batch b = c//2, head-group g = c%2 (6 heads each).
Per-core: projections for its 384 output dims + flash-style attention for
its 6 heads, all in transposed layouts so no P-matrix transposes are
needed:
  - hs^T [768, 2048] built via PE transposes
  - q_t/k_t [384, 2048] = W @ hs^T   (scores scale 1/8 and bias folded in)
  - v natural [2048, 384] via lhsT=hs^T, augmented with a ones column per
    head (x exp(attention_mask)) so one PV matmul yields numerator AND
    softmax denominator
  - S^T tiles [j=128, i<=512] straight from PE (2 heads packed in the
    64-row strips), exp on ACT, causal handled by block skipping + one
    128x128 triangle mask multiply on diagonal blocks
  - O^T [65, 512] accumulated in PSUM over j; PE-transposed back, divided
    by the denominator column, bias bv added, DMA'd out.
No max-subtraction is needed: scores are O(1) by construction and masked
entries are exactly zeroed multiplicatively.
"""

from contextlib import ExitStack

import numpy as np

import concourse.bacc as bacc
import concourse.bass as bass
import concourse.mybir as mybir
import concourse.tile as tile
from concourse.bass_utils import run_bass_kernel_spmd
from concourse.masks import make_identity, make_upper_triangular

B = 4
T = 2048
C = 768  # model dim (contraction for projections)
HD = 64
NHL = 6  # heads per core
HL = NHL * HD  # 384 local output dims
NT = T // 128  # 16 token tiles
NCB = C // 128  # 6 model-dim blocks
NMB = HL // 128  # 3 local d blocks
NIB = T // 512  # 4 query super-blocks
F32 = mybir.dt.float32
F32R = mybir.dt.float32r
MULT = mybir.AluOpType.mult
ADD = mybir.AluOpType.add
EXP = mybir.ActivationFunctionType.Exp

N_CORES = 8
_PROGRAM = None


def _r(ap):
    return ap.bitcast(F32R)


def build_program():
    nc = bacc.Bacc(
        "TRN2", target_bir_lowering=False, debug=False, num_devices=N_CORES
    )
    hs = nc.dram_tensor("hs", [T, C], F32, kind="ExternalInput").ap()
    wq = nc.dram_tensor("wq", [HL, C], F32, kind="ExternalInput").ap()
    wk = nc.dram_tensor("wk", [HL, C], F32, kind="ExternalInput").ap()
    wv = nc.dram_tensor("wv", [HL, C], F32, kind="ExternalInput").ap()
    bq = nc.dram_tensor("bq", [HL], F32, kind="ExternalInput").ap()
    bk = nc.dram_tensor("bk", [HL], F32, kind="ExternalInput").ap()
    bv = nc.dram_tensor("bv", [HL], F32, kind="ExternalInput").ap()
    am = nc.dram_tensor("am", [T], F32, kind="ExternalInput").ap()
    out = nc.dram_tensor("out", [T, HL], F32, kind="ExternalOutput").ap()

    with tile.TileContext(nc) as tc, ExitStack() as ctx:
        const = ctx.enter_context(tc.tile_pool(name="const", bufs=1))
        ident = const.tile([128, 128], F32, tag="ident")
        make_identity(nc, ident)
        tri = const.tile([128, 128], F32, tag="tri")
        make_upper_triangular(nc, tri, val=1.0, diag=True)  # tri[p,u]=1 if u>=p
        bq_s = const.tile([128, NMB], F32, tag="bq_s")
        bk_t = const.tile([128, NMB], F32, tag="bk_t")
        bv_bc = const.tile([128, HL], F32, tag="bv_bc")
        nc.sync.dma_start(out=bq_s, in_=bq.rearrange("(m p) -> p m", p=128))
        nc.sync.dma_start(out=bk_t, in_=bk.rearrange("(m p) -> p m", p=128))
        nc.sync.dma_start(
            out=bv_bc,
            in_=bass.AP(tensor=bv.tensor, offset=bv.offset, ap=[[0, 128], [1, HL]]),
        )
        # scale q-bias by 1/8 so it can fold into the score scaling
        nc.vector.tensor_scalar_mul(out=bq_s, in0=bq_s, scalar1=0.125)
        ones6 = const.tile([128, NHL], F32, tag="ones6")
        nc.vector.memset(ones6, 1.0)

        exp_am = []
        expp = ctx.enter_context(tc.tile_pool(name="expp", bufs=1))
        for ti in range(NT):
            ea = expp.tile([128, 1], F32, name=f"ea{ti}", tag=f"ea{ti}")
            amt = expp.tile([128, 1], F32, name=f"amt{ti}", tag=f"amt{ti}")
            nc.sync.dma_start(
                out=amt,
                in_=bass.AP(
                    tensor=am.tensor, offset=am.offset + 128 * ti, ap=[[1, 128], [1, 1]]
                ),
            )
            nc.scalar.activation(out=ea, in_=amt, func=EXP)
            exp_am.append(ea)

        # long-lived across B+C; opened before the A/B-scoped pools so pool
        # releases stay LIFO
        qkv = ctx.enter_context(tc.tile_pool(name="qkv", bufs=1))
        q_t = [qkv.tile([128, T], F32R, name=f"q_t{m}", tag=f"q_t{m}") for m in range(NMB)]
        k_t = [qkv.tile([128, T], F32R, name=f"k_t{m}", tag=f"k_t{m}") for m in range(NMB)]
        v_aug = [
            qkv.tile([128, NHL * (HD + 1)], F32R, name=f"va{ti}", tag=f"va{ti}")
            for ti in range(NT)
        ]

        # ---------------- phases A+B: transposes + projections -----------
        with ExitStack() as ab:
            hsT_p = ab.enter_context(tc.tile_pool(name="hsT_p", bufs=1))
            wT_p = ab.enter_context(tc.tile_pool(name="wT_p", bufs=1))
            psAB = ab.enter_context(tc.tile_pool(name="psAB", bufs=4, space="PSUM"))
            hsT = [
                hsT_p.tile([128, T], F32R, name=f"hsT{i}", tag=f"hsT{i}")
                for i in range(NCB)
            ]
            wT = {
                w: [
                    wT_p.tile([128, HL], F32R, name=f"wT{w}{i}", tag=f"wT{w}{i}")
                    for i in range(NCB)
                ]
                for w in ("q", "k", "v")
            }
            with tc.tile_pool(name="pa", bufs=3) as pa:
                for ti in range(NT):
                    hst = pa.tile([128, C], F32, name="hsl", tag="hsl")
                    nc.sync.dma_start(out=hst, in_=hs[128 * ti : 128 * (ti + 1), :])
                    for cb in range(NCB):
                        ps = psAB.tile([128, 128], F32, name="psa", tag="ps")
                        nc.tensor.transpose(
                            ps, hst[:, 128 * cb : 128 * (cb + 1)], ident
                        )
                        nc.vector.tensor_copy(
                            out=hsT[cb][:, 128 * ti : 128 * (ti + 1)], in_=ps
                        )
                for w, src in (("q", wq), ("k", wk), ("v", wv)):
                    for mt in range(NMB):
                        wt = pa.tile([128, C], F32, name="wl", tag="wl")
                        nc.sync.dma_start(
                            out=wt, in_=src[128 * mt : 128 * (mt + 1), :]
                        )
                        for cb in range(NCB):
                            ps = psAB.tile([128, 128], F32, name="psa", tag="ps")
                            nc.tensor.transpose(
                                ps, wt[:, 128 * cb : 128 * (cb + 1)], ident
                            )
                            nc.vector.tensor_copy(
                                out=wT[w][cb][:, 128 * mt : 128 * (mt + 1)], in_=ps
                            )

            for m in range(NMB):
                for nt in range(NIB):
                    tsl = slice(512 * nt, 512 * (nt + 1))
                    psq = psAB.tile([128, 512], F32, name="psb", tag="ps")
                    for kc in range(NCB):
                        nc.tensor.matmul(
                            psq,
                            lhsT=(wT["q"][kc][:, 128 * m : 128 * (m + 1)]),
                            rhs=(hsT[kc][:, tsl]),
                            start=(kc == 0),
                            stop=(kc == NCB - 1),
                        )
                    nc.vector.tensor_scalar(
                        out=q_t[m][:, tsl],
                        in0=psq,
                        scalar1=0.125,
                        scalar2=bq_s[:, m : m + 1],
                        op0=MULT,
                        op1=ADD,
                    )
                    psk = psAB.tile([128, 512], F32, name="psk", tag="ps")
                    for kc in range(NCB):
                        nc.tensor.matmul(
                            psk,
                            lhsT=(wT["k"][kc][:, 128 * m : 128 * (m + 1)]),
                            rhs=(hsT[kc][:, tsl]),
                            start=(kc == 0),
                            stop=(kc == NCB - 1),
                        )
                    nc.vector.tensor_scalar_add(
                        out=k_t[m][:, tsl], in0=psk, scalar1=bk_t[:, m : m + 1]
                    )
            for ti in range(NT):
                psv = psAB.tile([128, HL], F32, name="psv", tag="ps")
                for kc in range(NCB):
                    nc.tensor.matmul(
                        psv,
                        lhsT=(hsT[kc][:, 128 * ti : 128 * (ti + 1)]),
                        rhs=(wT["v"][kc]),
                        start=(kc == 0),
                        stop=(kc == NCB - 1),
                    )
                # rows scaled by exp(attention_mask[j]); per-head aug column
                # holds exp(am) so the PV matmul also yields the denominator
                va = v_aug[ti].rearrange("p (h x) -> p h x", x=HD + 1)
                nc.vector.tensor_scalar_mul(
                    out=va[:, :, 0:HD],
                    in0=psv.rearrange("p (h x) -> p h x", x=HD),
                    scalar1=exp_am[ti],
                )
                nc.vector.tensor_scalar_mul(
                    out=va[:, :, HD], in0=ones6, scalar1=exp_am[ti]
                )

        # ---------------- phase C: attention -----------------------------
        with ExitStack() as cctx:
            psC = cctx.enter_context(tc.tile_pool(name="psC", bufs=1, space="PSUM"))
            ptp = cctx.enter_context(tc.tile_pool(name="ptp", bufs=4))
            osbp = cctx.enter_context(tc.tile_pool(name="osbp", bufs=3))
            recp = cctx.enter_context(tc.tile_pool(name="recp", bufs=4))
            outp = cctx.enter_context(tc.tile_pool(name="outp", bufs=1))
            out_sb = [
                outp.tile([128, HL], F32, name=f"osb{ti}", tag=f"osb{ti}")
                for ti in range(NT)
            ]
            for pr in range(NHL // 2):
                for ib in range(NIB):
                    o_ps = [
                        psC.tile([65, 512], F32, name="o_ps", tag="o", bufs=2)
                        for _ in range(2)
                    ]
                    njb = 4 * (ib + 1)
                    for jb in range(njb):
                        off = max(0, 128 * jb - 512 * ib)
                        w = 512 - off
                        isl = slice(512 * ib + off, 512 * (ib + 1))
                        s_ps = [
                            psC.tile([128, 512], F32, name="s_ps", tag="s", bufs=4)
                            for _ in range(2)
                        ]
                        for h2 in range(2):
                            dsl = slice(64 * h2, 64 * (h2 + 1))
                            nc.tensor.matmul(
                                s_ps[h2][:, :w],
                                lhsT=(k_t[pr][dsl, 128 * jb : 128 * (jb + 1)]),
                                rhs=(q_t[pr][dsl, isl]),
                                start=True,
                                stop=True,
                            )
                        for h2 in range(2):
                            h = 2 * pr + h2
                            pt = ptp.tile([128, 512], F32R, name="pt", tag="pt")
                            nc.scalar.activation(
                                out=pt[:, :w], in_=s_ps[h2][:, :w], func=EXP
                            )
                            if jb >= 4 * ib:  # diagonal block: triangle mask
                                nc.vector.tensor_mul(
                                    out=pt[:, 0:128], in0=pt[:, 0:128], in1=tri
                                )
                            nc.tensor.matmul(
                                o_ps[h2][:, off:512],
                                lhsT=(v_aug[jb][:, 65 * h : 65 * h + 65]),
                                rhs=(pt[:, :w]),
                                start=(jb == 0),
                                stop=(jb == njb - 1),
                            )
                    for h2 in range(2):
                        h = 2 * pr + h2
                        osb = osbp.tile([65, 512], F32, name="osb_c", tag="osb_c")
                        nc.vector.tensor_copy(out=osb, in_=o_ps[h2])
                        for st in range(4):
                            i128 = 4 * ib + st
                            ptr = psC.tile([128, 65], F32, name="ptr", tag="tr", bufs=2)
                            nc.tensor.transpose(
                                ptr,
                                osb[:, 128 * st : 128 * (st + 1)],
                                ident[:65, :65],
                            )
                            rec = recp.tile([128, 1], F32, name="rec", tag="rec")
                            nc.vector.reciprocal(out=rec, in_=ptr[:, 64:65])
                            nc.vector.tensor_scalar_mul(
                                out=out_sb[i128][:, 64 * h : 64 * (h + 1)],
                                in0=ptr[:, 0:64],
                                scalar1=rec,
                            )
            for ti in range(NT):
                nc.vector.tensor_add(out=out_sb[ti], in0=out_sb[ti], in1=bv_bc)
                nc.sync.dma_start(
                    out=out[128 * ti : 128 * (ti + 1), :], in_=out_sb[ti]
                )

    nc.compile()
    return nc


def _get_program():
    global _PROGRAM
    if _PROGRAM is None:
        _PROGRAM = build_program()
    return _PROGRAM


def kernel(hidden_states, attention_mask, Wq, bq, Wk, bk, Wv, bv):
    nc = _get_program()
    f = lambda a: np.ascontiguousarray(np.asarray(a, dtype=np.float32))
    in_maps = []
    for c in range(N_CORES):
        b, g = c // 2, c % 2
        sl = slice(HL * g, HL * (g + 1))
        in_maps.append(
            {
                "hs": f(hidden_states[b]),
                "wq": f(Wq[sl]),
                "bq": f(bq[sl]),
                "wk": f(Wk[sl]),
                "bk": f(bk[sl]),
                "wv": f(Wv[sl]),
                "bv": f(bv[sl]),
                "am": f(attention_mask[b, 0, 0, :]),
            }
        )
    res = run_bass_kernel_spmd(nc, in_maps, list(range(N_CORES)))
    full = np.empty((B, T, 2 * HL), np.float32)
    for c in range(N_CORES):
        b, g = c // 2, c % 2
        full[b, :, HL * g : HL * (g + 1)] = res.results[c]["out"]
    return full
